# revision 28
# baseline (speedup 1.0000x reference)
"""Trainium2 Bass kernel for the NT-Xent / CLIP-style contrastive loss.

Reference computation (N=8192, D=512, fp32):
    zi_n, zj_n = row-normalize(z_i), row-normalize(z_j)
    sim = zi_n @ zj_n.T / TAU
    loss_e2t = mean_i( logsumexp_{j!=i}(sim[i,:]) - sim[i,i] )
    loss_t2e = mean_j( logsumexp_{i!=j}(sim[:,j]) - sim[j,j] )
    out = [ (loss_e2t+loss_t2e)/2, loss_e2t, loss_t2e ]

Sharding: rows of z_i are split across the 8 cores (1024 rows each); the
normalized z_j is replicated (the host plays the role of the all-gather).
Each core computes its [1024, 8192] tile of exp(sim); row sums feed
lse_row, 128-partial column sums feed lse_col, and the host finishes the
128-way + 8-core reduction plus the final log/mean epilogue.

The design goal is a never-stalling TensorE (the PE matmul stream is the
theoretical floor at ~55us/core). Each [128, 2048] column group per row
chunk is computed into TWO independent PSUM tiles so their consumers
release them separately (a single 4-bank tile would couple the PE to the
slowest consumer and cost ~15% PE idle):
  * gpA cols [0:1024]   -> ScalarE table exp (+fused accum_out row sums).
    The exp can start two matmuls before the burst ends, so gpA frees
    early. Cols [0:512] accumulate into colacc on VectorE (bf16 2x mode);
    cols [512:1024] ship to HBM as bf16 for host-side column sums.
  * gpB cols [1024:2048] -> VectorE Schraudolph fast exp -- a single
    tensor_scalar computing int16(round(x*A + B)) whose bit pattern IS
    the bf16 exp approximation (mean bias ~6e-5 after tuning B; the
    +-1.8%/elem sawtooth averages out across 1000+-element sums). The
    tile ships straight to HBM; the host reduces it into both row and
    column sums in fp64.
Splitting the exp across engines this way leaves ScalarE at ~65%,
VectorE at ~90%, and DMA at ~80% of the PE period.

Main matmul runs in fp8e4m3 with DoubleRow packing (2 contraction rows per
PE cell). Operands are scaled by 32 before the fp8 cast to stay clear of
denormals; the 1/32^2 is folded into the exp scale.
"""

import math
import os
import sys

for _p in ("/opt/trn_rl_repo", "/root/.axon_site/_ro/trn_rl_repo"):
    if os.path.isdir(_p) and _p not in sys.path:
        sys.path.insert(0, _p)

import numpy as np
import ml_dtypes

import concourse.bass as bass
import concourse.bacc as bacc
import concourse.mybir as mybir
import concourse.tile as tile
from concourse import bass_utils

TAU = 0.07
EPS = 1e-8

N = 8192            # batch
D = 512             # embed dim
NCORES = 8
NI = N // NCORES    # rows per core (1024)
P = 128             # partitions
RC = NI // P        # row chunks per core (8)
CCG = 2048          # columns per group (one iteration)
NCCG = N // CCG     # 4 groups
MMN = 512           # matmul moving size (one PSUM bank of fp32)
NS = RC * NCCG      # accum slots
HC = 1024           # columns per PSUM half-tile
KC = 1024           # colacc columns per group (device-accumulated)

FP8_SCALE = 32.0
# exp argument = psum * ES (psum carries the 32^2 fp8 pre-scale)
ES = 1.0 / (TAU * FP8_SCALE * FP8_SCALE)

# Schraudolph uint8/fp8e4m3 fast exp: fp8_bits(exp(y)) ~= round(y*8/ln2 + B)
# (3-bit mantissa -> 8 steps per octave; bias 7 -> 56 at y=0)
SCHRAUDOLPH_A = 8.0 / math.log(2.0) * ES
SCHRAUDOLPH_B = 56.0 - 0.46  # C=0.46 zeroes the mean bias (numpy scan)

BF16 = mybir.dt.bfloat16
F32 = mybir.dt.float32
FP8 = mybir.dt.float8e4
U8 = mybir.dt.uint8
NP_FP8 = mybir.dt.np(FP8)

LAST_RESULTS = None  # BassKernelResults of the most recent run (for test.py)

_compiled = {}


def _build():
    """Build + compile the single-core SPMD Bass program."""
    nc = bacc.Bacc("TRN2", target_bir_lowering=False, debug=False)

    # zi: [kk, p, slab, n] with contraction row d = kk*256 + slab*128 + p;
    # one whole-tensor DMA per kk matches the SBUF layout exactly (2KB
    # descriptor runs). zj chunks are contiguous per DMA start likewise.
    zi0_t = nc.dram_tensor("zi0_t", [2, P, 2, P], FP8, kind="ExternalInput")
    zir_t = nc.dram_tensor("zir_t", [2, P, 2, NI - P], FP8, kind="ExternalInput")
    zj0_t = nc.dram_tensor("zj0_t", [2, 2, P, 2, HC], FP8, kind="ExternalInput")
    zjr_t = nc.dram_tensor("zjr_t", [2, NCCG - 1, P, 2, CCG], FP8,
                           kind="ExternalInput")
    rows_d = nc.dram_tensor("rowsums", [P, NS], F32, kind="ExternalOutput")
    cols_d = nc.dram_tensor("colacc", [NCCG, P, KC], BF16, kind="ExternalOutput")
    etf_d = nc.dram_tensor("etf", [NCCG, RC, P, HC], U8, kind="ExternalOutput")

    with tile.TileContext(nc) as tc:
        _body(nc, tc, zi0_t.ap(), zir_t.ap(), zj0_t.ap(), zjr_t.ap(),
              rows_d.ap(), cols_d.ap(), etf_d.ap())

    nc.compile()
    return nc


def _body(nc, tc, zi0_t, zir_t, zj0_t, zjr_t, rows_d, cols_d, etf_d):
    from contextlib import ExitStack

    perf_mode = mybir.MatmulPerfMode.DoubleRow

    with ExitStack() as ctx:
        zpool = ctx.enter_context(tc.tile_pool(name="z", bufs=1))
        # 8 bufs: enough slack for exp tiles to wait out the input-DMA
        # window (~12us) before their outbound DMAs get ring time
        epool = ctx.enter_context(tc.tile_pool(name="e", bufs=12))
        apool = ctx.enter_context(tc.tile_pool(name="acc", bufs=1))
        psa = ctx.enter_context(
            tc.tile_pool(name="psa", bufs=2, space=bass.MemorySpace.PSUM)
        )
        psb = ctx.enter_context(
            tc.tile_pool(name="psb", bufs=2, space=bass.MemorySpace.PSUM)
        )

        # ---- PE clock warmup ------------------------------------------
        # ~10 dummy DoubleRow matmuls on a memset tile keep the PE busy
        # during the input DMA window so the HAM clock gate opens (1.2 ->
        # 2.4 GHz) before the first real matmul issues.
        wsrc = zpool.tile([P, 2, MMN], FP8, tag="wsrc", name="wsrc")
        nc.gpsimd.memset(wsrc[:], 0)
        wp = psa.tile([P, HC], F32, tag="GA", name="warm")
        for w in range(10):
            nc.tensor.matmul(
                wp[:, 0:MMN],
                wsrc[:, :, 0:P],
                wsrc[:],
                start=True,
                stop=True,
                perf_mode=perf_mode,
            )

        # ---- stage inputs in SBUF -------------------------------------
        # Two HWDGE rings (sync + scalar) dispatch concurrently; order the
        # transfers by when the PE consumes them. Group 0 is cc-sliced so
        # the first matmuls start as soon as ~300KB have landed.
        zi_sb = [
            zpool.tile([P, 2, NI], FP8, tag=f"zi{k}", name=f"zi{k}")
            for k in range(2)
        ]
        zj_sb = [
            zpool.tile([P, 2, N], FP8, tag=f"zj{k}", name=f"zj{k}")
            for k in range(2)
        ]
        # All input starts ride the sync ring (contiguous sources, 2KB
        # descriptor runs); the scalar ring must stay clear so EXP_0 isn't
        # stuck behind input dispatches (~650ns each + ring-full gaps).
        for k in range(2):  # row chunk 0, 32KB contiguous each
            nc.sync.dma_start(zi_sb[k][:, :, 0:P], zi0_t[k])
        for h in range(2):  # group 0 in halves (gpA cols first), 256KB
            for k in range(2):
                nc.sync.dma_start(
                    zj_sb[k][:, :, h * HC:(h + 1) * HC], zj0_t[k, h]
                )
        for k in range(2):  # row chunks 1-7, 224KB contiguous each
            nc.sync.dma_start(zi_sb[k][:, :, P:NI], zir_t[k])
        for g in range(1, NCCG):  # groups 1..3, 512KB contiguous each
            for k in range(2):
                nc.sync.dma_start(
                    zj_sb[k][:, :, g * CCG:(g + 1) * CCG], zjr_t[k, g - 1]
                )

        colacc = apool.tile([P, NCCG * KC], BF16, tag="colacc")
        rows_sb = apool.tile([P, NS], F32, tag="rows")

        # ---- main loop ------------------------------------------------
        for g in range(NCCG):
            c0 = g * CCG
            for rc in range(RC):
                slot = rc * NCCG + g
                gpa = psa.tile([P, HC], F32, tag="GA")
                gpb = psb.tile([P, HC], F32, tag="GB")
                for k in range(2):
                    lhsT = zi_sb[k][:, :, rc * P:(rc + 1) * P]
                    for cc in range(CCG // MMN):
                        gp = gpa if cc < 2 else gpb
                        o = (cc % 2) * MMN
                        rhs = zj_sb[k][:, :, c0 + cc * MMN:c0 + (cc + 1) * MMN]
                        nc.tensor.matmul(
                            gp[:, o:o + MMN],
                            lhsT,
                            rhs,
                            start=(k == 0),
                            stop=(k == 1),
                            perf_mode=perf_mode,
                        )

                # fast exp of gpB -> its own tile, shipped to the host
                etf = epool.tile([P, HC], U8, tag="EF")
                nc.vector.tensor_scalar(
                    etf[:],
                    gpb[:],
                    SCHRAUDOLPH_A,
                    SCHRAUDOLPH_B,
                    mybir.AluOpType.mult,
                    mybir.AluOpType.add,
                )
                # sync ring: its dispatch waits on the schrd sem, which
                # would stall EXP dispatch if it sat on the scalar ring
                nc.sync.dma_start(etf_d[g, rc, :, :], etf[:])

                # table exp of gpA + fused row-partials; rc==0 writes
                # colacc directly (no add needed)
                a0 = g * KC
                if rc == 0:
                    nc.scalar.activation(
                        colacc[:, a0:a0 + KC],
                        gpa[:],
                        mybir.ActivationFunctionType.Exp,
                        bias=0.0,
                        scale=ES,
                        accum_out=rows_sb[:, slot:slot + 1],
                    )
                else:
                    et = epool.tile([P, HC], BF16, tag="E")
                    nc.scalar.activation(
                        et[:],
                        gpa[:],
                        mybir.ActivationFunctionType.Exp,
                        bias=0.0,
                        scale=ES,
                        accum_out=rows_sb[:, slot:slot + 1],
                    )
                    # accumulate split across VectorE / GpSimd
                    nc.vector.tensor_add(
                        colacc[:, a0:a0 + KC // 2],
                        colacc[:, a0:a0 + KC // 2],
                        et[:, 0:KC // 2],
                    )
                    nc.gpsimd.tensor_add(
                        colacc[:, a0 + KC // 2:a0 + KC],
                        colacc[:, a0 + KC // 2:a0 + KC],
                        et[:, KC // 2:KC],
                    )
            # this group's accumulated columns -- ship while next runs
            nc.sync.dma_start(cols_d[g, :, :], colacc[:, g * KC:(g + 1) * KC])

        nc.sync.dma_start(rows_d[:, :], rows_sb[:])


def _get_nc():
    if "nc" not in _compiled:
        _compiled["nc"] = _build()
    return _compiled["nc"]


def _pack_fp8(zt):
    """[D, n] fp32 -> [2, 128, 2, n] fp8 with d = kk*256 + slab*128 + p."""
    n = zt.shape[1]
    return np.ascontiguousarray(
        (zt * FP8_SCALE).reshape(2, 2, P, n).transpose(0, 2, 1, 3)
    ).astype(NP_FP8)


def _pack_fp8_zj(zt):
    """[D, N] fp32 -> ([2, 2, P, 2, HC] g0 h-split, [2, NCCG-1, P, 2, CCG])
    fp8 with d = kk*256 + slab*128 + p. Every DMA-start source chunk is
    contiguous so descriptors run at the full 2KB size."""
    q = (zt * FP8_SCALE).reshape(2, 2, P, NCCG, CCG)
    zj0 = np.ascontiguousarray(
        q[:, :, :, 0, :].reshape(2, 2, P, 2, HC).transpose(0, 3, 2, 1, 4)
    ).astype(NP_FP8)
    zjr = np.ascontiguousarray(
        q[:, :, :, 1:, :].transpose(0, 3, 2, 1, 4)
    ).astype(NP_FP8)
    return zj0, zjr


def _prep_inputs(z_i, z_j):
    """Host-side sharding: normalize (fp32, as the reference), transpose to
    [D, N] (the layout the PE contracts over), quantize, slice per core."""
    zi = np.asarray(z_i, dtype=np.float32)
    zj = np.asarray(z_j, dtype=np.float32)
    ni = np.maximum(np.sqrt((zi * zi).sum(-1, keepdims=True)), EPS)
    nj = np.maximum(np.sqrt((zj * zj).sum(-1, keepdims=True)), EPS)
    zin = zi / ni
    zjn = zj / nj
    pos = (zin * zjn).sum(-1, dtype=np.float64) / TAU  # diagonal of sim, [N]

    zin_t = zin.T  # [D, N]
    zjn_t = zjn.T

    in_maps = []
    zj0, zjr = _pack_fp8_zj(zjn_t)
    for c in range(NCORES):
        zi_pack = _pack_fp8(zin_t[:, c * NI:(c + 1) * NI])
        in_maps.append(
            {
                "zi0_t": np.ascontiguousarray(zi_pack[:, :, :, 0:P]),
                "zir_t": np.ascontiguousarray(zi_pack[:, :, :, P:NI]),
                "zj0_t": zj0,
                "zjr_t": zjr,
            }
        )
    return in_maps, pos


def _reduce_core(out):
    """Device outputs of one core -> (rowsum[NI], colsum[N]) in fp64."""
    rows = out["rowsums"].astype(np.float64)          # [128, NS], cols [0:HC]
    etf = out["etf"].view(ml_dtypes.float8_e4m3fn).astype(np.float64)
    # rowsum: scalar-slice accum per (rc, g) + host-reduced fast slice
    per_rc = rows.reshape(P, RC, NCCG).sum(-1)        # [p, rc]
    per_rc = per_rc + etf.sum(-1).sum(0).T            # fast slice, [p, rc]
    rowsum = per_rc.T.reshape(-1)                     # global row = rc*128+p
    # colsum: device colacc for [0:KC), host etf for the rest
    colacc = out["colacc"].astype(np.float64)         # [NCCG, 128, KC]
    colsum = np.empty(N, dtype=np.float64)
    cview = colsum.reshape(NCCG, CCG)
    cview[:, 0:KC] = colacc.sum(1)
    cview[:, KC:] = etf.sum((1, 2))
    return rowsum, colsum


def kernel(z_i, z_j):
    global LAST_RESULTS
    in_maps, pos = _prep_inputs(z_i, z_j)
    nc = _get_nc()

    res = bass_utils.run_bass_kernel_spmd(nc, in_maps, core_ids=list(range(NCORES)))
    LAST_RESULTS = res

    rowsum = np.zeros(N, dtype=np.float64)
    colsum = np.zeros(N, dtype=np.float64)
    for c in range(NCORES):
        r, cs = _reduce_core(res.results[c])
        rowsum[c * NI:(c + 1) * NI] = r
        colsum += cs

    # host-side "all-reduce" epilogue: drop the diagonal, logs, means
    exp_pos = np.exp(pos)
    lse_row = np.log(rowsum - exp_pos)
    lse_col = np.log(colsum - exp_pos)
    loss_e2t = np.mean(lse_row - pos)
    loss_t2e = np.mean(lse_col - pos)
    loss = 0.5 * (loss_e2t + loss_t2e)
    return np.stack([loss, loss_e2t, loss_t2e]).astype(np.float32)


# revision 29
# speedup vs baseline: 1.0045x; 1.0045x over previous
"""Trainium2 Bass kernel for the NT-Xent / CLIP-style contrastive loss.

Reference computation (N=8192, D=512, fp32):
    zi_n, zj_n = row-normalize(z_i), row-normalize(z_j)
    sim = zi_n @ zj_n.T / TAU
    loss_e2t = mean_i( logsumexp_{j!=i}(sim[i,:]) - sim[i,i] )
    loss_t2e = mean_j( logsumexp_{i!=j}(sim[:,j]) - sim[j,j] )
    out = [ (loss_e2t+loss_t2e)/2, loss_e2t, loss_t2e ]

Sharding: rows of z_i are split across the 8 cores (1024 rows each); the
normalized z_j is replicated (the host plays the role of the all-gather).
Each core computes its [1024, 8192] tile of exp(sim); row sums feed
lse_row, 128-partial column sums feed lse_col, and the host finishes the
128-way + 8-core reduction plus the final log/mean epilogue.

The design goal is a never-stalling TensorE (the PE matmul stream is the
theoretical floor at ~55us/core). Each [128, 2048] column group per row
chunk is computed into TWO independent PSUM tiles so their consumers
release them separately (a single 4-bank tile would couple the PE to the
slowest consumer and cost ~15% PE idle):
  * gpA cols [0:1024]   -> ScalarE table exp (+fused accum_out row sums).
    The exp can start two matmuls before the burst ends, so gpA frees
    early. Cols [0:512] accumulate into colacc on VectorE (bf16 2x mode);
    cols [512:1024] ship to HBM as bf16 for host-side column sums.
  * gpB cols [1024:2048] -> VectorE Schraudolph fast exp -- a single
    tensor_scalar computing int16(round(x*A + B)) whose bit pattern IS
    the bf16 exp approximation (mean bias ~6e-5 after tuning B; the
    +-1.8%/elem sawtooth averages out across 1000+-element sums). The
    tile ships straight to HBM; the host reduces it into both row and
    column sums in fp64.
Splitting the exp across engines this way leaves ScalarE at ~65%,
VectorE at ~90%, and DMA at ~80% of the PE period.

Main matmul runs in fp8e4m3 with DoubleRow packing (2 contraction rows per
PE cell). Operands are scaled by 32 before the fp8 cast to stay clear of
denormals; the 1/32^2 is folded into the exp scale.
"""

import math
import os
import sys

for _p in ("/opt/trn_rl_repo", "/root/.axon_site/_ro/trn_rl_repo"):
    if os.path.isdir(_p) and _p not in sys.path:
        sys.path.insert(0, _p)

import numpy as np
import ml_dtypes

import concourse.bass as bass
import concourse.bacc as bacc
import concourse.mybir as mybir
import concourse.tile as tile
from concourse import bass_utils

TAU = 0.07
EPS = 1e-8

N = 8192            # batch
D = 512             # embed dim
NCORES = 8
NI = N // NCORES    # rows per core (1024)
P = 128             # partitions
RC = NI // P        # row chunks per core (8)
CCG = 2048          # columns per group (one iteration)
NCCG = N // CCG     # 4 groups
MMN = 512           # matmul moving size (one PSUM bank of fp32)
NS = RC * NCCG      # accum slots
HC = 1024           # columns per PSUM half-tile
KC = 1024           # colacc columns per group (device-accumulated)
KCV = 640           # colacc columns accumulated on VectorE
KCP = KC - KCV      # colacc columns accumulated on GpSimd (own tile)

FP8_SCALE = 32.0
# exp argument = psum * ES (psum carries the 32^2 fp8 pre-scale)
ES = 1.0 / (TAU * FP8_SCALE * FP8_SCALE)

# Schraudolph uint8/fp8e4m3 fast exp: fp8_bits(exp(y)) ~= round(y*8/ln2 + B)
# (3-bit mantissa -> 8 steps per octave; bias 7 -> 56 at y=0)
SCHRAUDOLPH_A = 8.0 / math.log(2.0) * ES
SCHRAUDOLPH_B = 56.0 - 0.46  # C=0.46 zeroes the mean bias (numpy scan)

BF16 = mybir.dt.bfloat16
F32 = mybir.dt.float32
FP8 = mybir.dt.float8e4
U8 = mybir.dt.uint8
NP_FP8 = mybir.dt.np(FP8)

LAST_RESULTS = None  # BassKernelResults of the most recent run (for test.py)

_compiled = {}


def _build():
    """Build + compile the single-core SPMD Bass program."""
    nc = bacc.Bacc("TRN2", target_bir_lowering=False, debug=False)

    # zi: [kk, p, slab, n] with contraction row d = kk*256 + slab*128 + p;
    # one whole-tensor DMA per kk matches the SBUF layout exactly (2KB
    # descriptor runs). zj chunks are contiguous per DMA start likewise.
    zi_t = nc.dram_tensor("zi_t", [RC, P, 2, 2, P], FP8, kind="ExternalInput")
    zj0_t = nc.dram_tensor("zj0_t", [2, 2, P, 2, HC], FP8, kind="ExternalInput")
    zjr_t = nc.dram_tensor("zjr_t", [2, NCCG - 1, P, 2, CCG], FP8,
                           kind="ExternalInput")
    rows_d = nc.dram_tensor("rowsums", [P, NS], F32, kind="ExternalOutput")
    cols_d = nc.dram_tensor("colacc", [NCCG, P, KC], BF16, kind="ExternalOutput")
    etf_d = nc.dram_tensor("etf", [NCCG, RC, P, HC], U8, kind="ExternalOutput")

    with tile.TileContext(nc) as tc:
        _body(nc, tc, zi_t.ap(), zj0_t.ap(), zjr_t.ap(),
              rows_d.ap(), cols_d.ap(), etf_d.ap())

    nc.compile()
    return nc


def _body(nc, tc, zi_t, zj0_t, zjr_t, rows_d, cols_d, etf_d):
    from contextlib import ExitStack

    perf_mode = mybir.MatmulPerfMode.DoubleRow

    with ExitStack() as ctx:
        zpool = ctx.enter_context(tc.tile_pool(name="z", bufs=1))
        # 8 bufs: enough slack for exp tiles to wait out the input-DMA
        # window (~12us) before their outbound DMAs get ring time
        epool = ctx.enter_context(tc.tile_pool(name="e", bufs=12))
        apool = ctx.enter_context(tc.tile_pool(name="acc", bufs=1))
        psa = ctx.enter_context(
            tc.tile_pool(name="psa", bufs=2, space=bass.MemorySpace.PSUM)
        )
        psb = ctx.enter_context(
            tc.tile_pool(name="psb", bufs=2, space=bass.MemorySpace.PSUM)
        )

        # ---- PE clock warmup ------------------------------------------
        # ~10 dummy DoubleRow matmuls on a memset tile keep the PE busy
        # during the input DMA window so the HAM clock gate opens (1.2 ->
        # 2.4 GHz) before the first real matmul issues.
        wsrc = zpool.tile([P, 2, MMN], FP8, tag="wsrc", name="wsrc")
        nc.gpsimd.memset(wsrc[:], 0)
        wp = psa.tile([P, HC], F32, tag="GA", name="warm")
        for w in range(10):
            nc.tensor.matmul(
                wp[:, 0:MMN],
                wsrc[:, :, 0:P],
                wsrc[:],
                start=True,
                stop=True,
                perf_mode=perf_mode,
            )

        # ---- stage inputs in SBUF -------------------------------------
        # Two HWDGE rings (sync + scalar) dispatch concurrently; order the
        # transfers by when the PE consumes them. Group 0 is cc-sliced so
        # the first matmuls start as soon as ~300KB have landed.
        zi_sb = zpool.tile([P, 2, 2, NI], FP8, tag="zi", name="zi")
        zj_sb = [
            zpool.tile([P, 2, N], FP8, tag=f"zj{k}", name=f"zj{k}")
            for k in range(2)
        ]
        # All input starts ride the sync ring (contiguous sources, 2KB
        # descriptor runs); the scalar ring must stay clear so EXP_0 isn't
        # stuck behind input dispatches (~650ns each + ring-full gaps).
        def _zi(r0, r1):  # contiguous [r, p, k, s, c] chunks, 64KB/rc
            nc.sync.dma_start(
                zi_sb[:, :, :, r0 * P:r1 * P].transpose([3, 0, 1, 2])
                if False else zi_sb[:, :, :, r0 * P:r1 * P],
                zi_t[r0] if r1 == r0 + 1 else zi_t[r0:r1],
            )

        _zi(0, 1)
        for h in range(2):  # group 0 in halves (gpA cols first), 256KB
            for k in range(2):
                nc.sync.dma_start(
                    zj_sb[k][:, :, h * HC:(h + 1) * HC], zj0_t[k, h]
                )
        for r in range(1, 5):  # row chunks 1-4 just in time
            _zi(r, r + 1)
        for k in range(2):  # group 1
            nc.sync.dma_start(zj_sb[k][:, :, CCG:2 * CCG], zjr_t[k, 0])
        for r in range(5, RC):  # row chunks 5-7
            _zi(r, r + 1)
        for g in range(2, NCCG):  # groups 2..3, 512KB contiguous each
            for k in range(2):
                nc.sync.dma_start(
                    zj_sb[k][:, :, g * CCG:(g + 1) * CCG], zjr_t[k, g - 1]
                )

        colacc = apool.tile([P, NCCG * KCV], BF16, tag="colacc")
        colach = apool.tile([P, NCCG * KCP], BF16, tag="colach")
        rows_sb = apool.tile([P, NS], F32, tag="rows")

        # ---- main loop ------------------------------------------------
        for g in range(NCCG):
            c0 = g * CCG
            for rc in range(RC):
                slot = rc * NCCG + g
                gpa = psa.tile([P, HC], F32, tag="GA")
                gpb = psb.tile([P, HC], F32, tag="GB")
                for k in range(2):
                    lhsT = zi_sb[:, k, :, rc * P:(rc + 1) * P]
                    for cc in range(CCG // MMN):
                        gp = gpa if cc < 2 else gpb
                        o = (cc % 2) * MMN
                        rhs = zj_sb[k][:, :, c0 + cc * MMN:c0 + (cc + 1) * MMN]
                        nc.tensor.matmul(
                            gp[:, o:o + MMN],
                            lhsT,
                            rhs,
                            start=(k == 0),
                            stop=(k == 1),
                            perf_mode=perf_mode,
                        )

                # fast exp of gpB -> its own tile, shipped to the host
                etf = epool.tile([P, HC], U8, tag="EF")
                nc.vector.tensor_scalar(
                    etf[:],
                    gpb[:],
                    SCHRAUDOLPH_A,
                    SCHRAUDOLPH_B,
                    mybir.AluOpType.mult,
                    mybir.AluOpType.add,
                )
                # sync ring: its dispatch waits on the schrd sem, which
                # would stall EXP dispatch if it sat on the scalar ring
                nc.sync.dma_start(etf_d[g, rc, :, :], etf[:])

                # table exp of gpA + fused row-partials
                et = epool.tile([P, HC], BF16, tag="E")
                nc.scalar.activation(
                    et[:],
                    gpa[:],
                    mybir.ActivationFunctionType.Exp,
                    bias=0.0,
                    scale=ES,
                    accum_out=rows_sb[:, slot:slot + 1],
                )
                av = g * KCV
                ah = g * KCP
                if rc == 0:
                    nc.vector.tensor_copy(colacc[:, av:av + KCV], et[:, 0:KCV])
                    nc.gpsimd.tensor_copy(colach[:, ah:ah + KCP],
                                          et[:, KCV:KC])
                else:
                    # accumulate split across VectorE / GpSimd (own tiles)
                    nc.vector.tensor_add(
                        colacc[:, av:av + KCV],
                        colacc[:, av:av + KCV],
                        et[:, 0:KCV],
                    )
                    nc.gpsimd.tensor_add(
                        colach[:, ah:ah + KCP],
                        colach[:, ah:ah + KCP],
                        et[:, KCV:KC],
                    )
            # this group's accumulated columns -- ship while next runs
            nc.sync.dma_start(cols_d[g, :, 0:KCV],
                              colacc[:, g * KCV:(g + 1) * KCV])
            nc.sync.dma_start(cols_d[g, :, KCV:KC],
                              colach[:, g * KCP:(g + 1) * KCP])

        nc.sync.dma_start(rows_d[:, :], rows_sb[:])


def _get_nc():
    if "nc" not in _compiled:
        _compiled["nc"] = _build()
    return _compiled["nc"]


def _pack_fp8(zt):
    """[D, n] fp32 -> [rc, 128, 2, 2, 128] fp8 with d = kk*256 + slab*128
    + p and col = rc*128 + c; each rc chunk is contiguous (64KB)."""
    n = zt.shape[1]
    return np.ascontiguousarray(
        (zt * FP8_SCALE).reshape(2, 2, P, n // P, P).transpose(3, 2, 0, 1, 4)
    ).astype(NP_FP8)


def _pack_fp8_zj(zt):
    """[D, N] fp32 -> ([2, 2, P, 2, HC] g0 h-split, [2, NCCG-1, P, 2, CCG])
    fp8 with d = kk*256 + slab*128 + p. Every DMA-start source chunk is
    contiguous so descriptors run at the full 2KB size."""
    q = (zt * FP8_SCALE).reshape(2, 2, P, NCCG, CCG)
    zj0 = np.ascontiguousarray(
        q[:, :, :, 0, :].reshape(2, 2, P, 2, HC).transpose(0, 3, 2, 1, 4)
    ).astype(NP_FP8)
    zjr = np.ascontiguousarray(
        q[:, :, :, 1:, :].transpose(0, 3, 2, 1, 4)
    ).astype(NP_FP8)
    return zj0, zjr


def _prep_inputs(z_i, z_j):
    """Host-side sharding: normalize (fp32, as the reference), transpose to
    [D, N] (the layout the PE contracts over), quantize, slice per core."""
    zi = np.asarray(z_i, dtype=np.float32)
    zj = np.asarray(z_j, dtype=np.float32)
    ni = np.maximum(np.sqrt((zi * zi).sum(-1, keepdims=True)), EPS)
    nj = np.maximum(np.sqrt((zj * zj).sum(-1, keepdims=True)), EPS)
    zin = zi / ni
    zjn = zj / nj
    pos = (zin * zjn).sum(-1, dtype=np.float64) / TAU  # diagonal of sim, [N]

    zin_t = zin.T  # [D, N]
    zjn_t = zjn.T

    in_maps = []
    zj0, zjr = _pack_fp8_zj(zjn_t)
    for c in range(NCORES):
        in_maps.append(
            {
                "zi_t": _pack_fp8(zin_t[:, c * NI:(c + 1) * NI]),
                "zj0_t": zj0,
                "zjr_t": zjr,
            }
        )
    return in_maps, pos


def _reduce_core(out):
    """Device outputs of one core -> (rowsum[NI], colsum[N]) in fp64."""
    rows = out["rowsums"].astype(np.float64)          # [128, NS], cols [0:HC]
    etf = out["etf"].view(ml_dtypes.float8_e4m3fn).astype(np.float64)
    # rowsum: scalar-slice accum per (rc, g) + host-reduced fast slice
    per_rc = rows.reshape(P, RC, NCCG).sum(-1)        # [p, rc]
    per_rc = per_rc + etf.sum(-1).sum(0).T            # fast slice, [p, rc]
    rowsum = per_rc.T.reshape(-1)                     # global row = rc*128+p
    # colsum: device colacc for [0:KC), host etf for the rest
    colacc = out["colacc"].astype(np.float64)         # [NCCG, 128, KC]
    colsum = np.empty(N, dtype=np.float64)
    cview = colsum.reshape(NCCG, CCG)
    cview[:, 0:KC] = colacc.sum(1)
    cview[:, KC:] = etf.sum((1, 2))
    return rowsum, colsum


def kernel(z_i, z_j):
    global LAST_RESULTS
    in_maps, pos = _prep_inputs(z_i, z_j)
    nc = _get_nc()

    res = bass_utils.run_bass_kernel_spmd(nc, in_maps, core_ids=list(range(NCORES)))
    LAST_RESULTS = res

    rowsum = np.zeros(N, dtype=np.float64)
    colsum = np.zeros(N, dtype=np.float64)
    for c in range(NCORES):
        r, cs = _reduce_core(res.results[c])
        rowsum[c * NI:(c + 1) * NI] = r
        colsum += cs

    # host-side "all-reduce" epilogue: drop the diagonal, logs, means
    exp_pos = np.exp(pos)
    lse_row = np.log(rowsum - exp_pos)
    lse_col = np.log(colsum - exp_pos)
    loss_e2t = np.mean(lse_row - pos)
    loss_t2e = np.mean(lse_col - pos)
    loss = 0.5 * (loss_e2t + loss_t2e)
    return np.stack([loss, loss_e2t, loss_t2e]).astype(np.float32)
